# revision 1
# baseline (speedup 1.0000x reference)
"""Trainium2 Bass kernel for MultiHeadFAVORAttention (Performer, causal).

Sharding: 8 cores = 4 batches x 2 head-groups (4 heads each).
Algorithm: chunked linear attention (chunk C=128) -- the causal scan over
L=2048 becomes per-chunk matmuls:
  A~[j,i]   = sum_m kp[j,m] qp[i,m]          (masked j<=i, intra-chunk)
  num'[i]   = maskedA~.T @ V' + QP.T @ S'    (V' has a ones column -> den)
  S'       += KP.T @ V'                      (PSUM-resident running state)
  attn      = num/den; out = attnT.T @ Wo    (partial; host sums head-groups)

fp8 (e4m3) DoubleRow perf mode (0.5 cyc/row) is used for the PE-heavy
matmuls that tolerate it (measured on the real inputs):
  - QKV input projections: x and W split hi+lo fp8 (x*16, W*64; product
    scale 1024); 3 cross products, lo*lo dropped (~0.1% error).
  - A~: qp/kp feature tiles stored fp8 at scale 1024 (PSUM 1024^2*A).
  - num-inter: qp8 (1024) x S8 (8) -> num PSUM at 8192*true; the intra
    path matches via the mask constant 8192/1024^2 = 1/128.
dS / num-intra / out-projection stay bf16 (fp8 there fails the 2e-2 gate).

Act/DVE evacuations are merged into 1024-col (2-bank) PSUM tiles to halve
per-instruction overhead. All projections, features and kp tiles are
precomputed in a DMA-paced prologue; the scan phase then only needs its
per-chunk mask/divide/state evacuations on the vector engines.
"""
import math

import numpy as np
import ml_dtypes

import concourse.bass as bass
import concourse.mybir as mybir
import concourse.tile as tile
from concourse import bacc, bass_utils

# ---------------------------------------------------------------- constants
B, L, DIN = 4, 2048, 512
HEADS, D, M = 8, 64, 256
NH = 4            # heads per core
C = 128           # scan chunk
NCH = L // C      # 16 chunks
NW = 4            # chunks per feature window (window = 512 cols)
STAB = 1e-5
RATIO = 1.0 / math.sqrt(M)
N_CORES = 8

_F32 = mybir.dt.float32
_BF16 = mybir.dt.bfloat16
_F8 = mybir.dt.float8e4
_NP_F8 = ml_dtypes.float8_e4m3
_NP_BF16 = ml_dtypes.bfloat16
_DR = mybir.MatmulPerfMode.DoubleRow

SC_X = 16.0       # x fp8 scale
SC_W = 64.0       # W fp8 scale
SC_QK = SC_X * SC_W          # QKT psum scale (1024)
SC_F = 1024.0     # qp/kp fp8 scale (prj constants pre-scaled by SC_F)
SC_S = 8.0        # S fp8 scale
SC_NUM = SC_F * SC_S         # num psum scale (8192)
MASKC = SC_NUM / (SC_F * SC_F)  # 1/128: A~ psum -> mA at SC_NUM units

# cdt (bf16) column offsets
_OFF_PRJ = 0      # prjE|prjO, pre-scaled by SC_F  (512 cols)
_OFF_WO = 512     # out-proj weights (1024)
_OFF_ID = 1536    # identity for PE transposes (128)
_W_CDT = 1664
# cf32: mask*MASKC 0:128, feat bias q 128:136, feat bias k 136:144
_W_CF = 144
_CACHED = {}


def _build_nc():
    """Build the SPMD Bass program (identical on all 8 cores)."""
    nc = bacc.Bacc("TRN2", target_bir_lowering=False, debug=False,
                   num_devices=N_CORES)

    xq8 = nc.dram_tensor("xq8", [DIN, L], _F8, kind="ExternalInput").ap()
    xk8 = nc.dram_tensor("xk8", [DIN, L], _F8, kind="ExternalInput").ap()
    xv8 = nc.dram_tensor("xv8", [2, DIN, L], _F8, kind="ExternalInput").ap()
    cfp8 = nc.dram_tensor("cfp8", [128, 6144], _F8, kind="ExternalInput").ap()
    cdt = nc.dram_tensor("cdt", [128, _W_CDT], _BF16, kind="ExternalInput").ap()
    cf32 = nc.dram_tensor("cf32", [128, _W_CF], _F32, kind="ExternalInput").ap()
    outp = nc.dram_tensor("outp", [L, 512], _BF16, kind="ExternalOutput").ap()

    ACT = mybir.ActivationFunctionType
    ALU = mybir.AluOpType

    with tile.TileContext(nc) as tc:
        with (
            tc.tile_pool(name="const", bufs=1) as const,
            tc.tile_pool(name="xp", bufs=1) as xp,
            tc.tile_pool(name="qk", bufs=1) as qk,
            tc.tile_pool(name="vp", bufs=1) as vpool,
            tc.tile_pool(name="featq", bufs=4) as featq,
            tc.tile_pool(name="featk", bufs=4) as featk,
            tc.tile_pool(name="kpp", bufs=16) as kpp,
            tc.tile_pool(name="small", bufs=10) as small,
            tc.tile_pool(name="att", bufs=1) as att,
            tc.tile_pool(name="outs", bufs=4) as outs,
            tc.tile_pool(name="psBig", bufs=2, space="PSUM") as psBig,
            tc.tile_pool(name="psA", bufs=2, space="PSUM") as psA,
            tc.tile_pool(name="psS", bufs=1, space="PSUM") as psS,
        ):
            # ---------------- DMA order: wq/wk fp8 first, then the window-0
            # activations, so the first QKT matmuls start ASAP.
            c8_sb = const.tile([128, 6144], _F8)
            nc.sync.dma_start(c8_sb[:, 0:4096], cfp8[:, 0:4096])

            xq_sb = xp.tile([128, 4, L], _F8, tag="xq")
            xk_sb = xp.tile([128, 4, L], _F8, tag="xk")
            xv_sb = xp.tile([128, 2, 4, L], _F8, tag="xv")
            srcs = {
                "q": (xq_sb, xq8.rearrange("(ko p) l -> p ko l", p=128)),
                "k": (xk_sb, xk8.rearrange("(ko p) l -> p ko l", p=128)),
                "v": (xv_sb, xv8.rearrange("t (ko p) l -> p t ko l", p=128)),
            }

            def dma_quarter(nm, nt):
                x_sb, src = srcs[nm]
                nc.sync.dma_start(x_sb[..., nt * 512:(nt + 1) * 512],
                                  src[..., nt * 512:(nt + 1) * 512])

            dma_quarter("q", 0)
            dma_quarter("k", 0)
            cdt_sb = const.tile([128, _W_CDT], _BF16)
            nc.sync.dma_start(cdt_sb[:], cdt[:])
            cf_sb = const.tile([128, _W_CF], _F32)
            nc.sync.dma_start(cf_sb[:], cf32[:])
            stab_sb = const.tile([128, 1], _F32)
            nc.vector.memset(stab_sb[:], STAB)
            dma_quarter("v", 0)
            nc.sync.dma_start(c8_sb[:, 4096:], cfp8[:, 4096:])
            for nt in range(1, 4):
                for nm in ("q", "k", "v"):
                    dma_quarter(nm, nt)

            w8 = c8_sb.rearrange("p (w ko x) -> p w ko x", w=6, ko=4)
            prj_sb = cdt_sb[:, _OFF_PRJ:_OFF_PRJ + 512]   # [prjE|prjO]*SC_F
            prjE_sb = cdt_sb[:, _OFF_PRJ:_OFF_PRJ + 256]
            prjO_sb = cdt_sb[:, _OFF_PRJ + 256:_OFF_PRJ + 512]
            wo_sb = cdt_sb[:, _OFF_WO:_OFF_WO + 1024].rearrange(
                "p (mh x) -> p mh x", mh=2)
            id_sb = cdt_sb[:, _OFF_ID:_OFF_ID + 128]
            mask_sb = cf_sb[:, 0:128]

            def fbias(qk_i, h, mh):
                col = 128 + qk_i * 8 + h * 2 + mh
                return cf_sb[:, col:col + 1]

            # ---------------- QT / KT projections (fp8 hi/lo DoubleRow),
            # merged evac: both mt halves of one tensor-window in one instr
            QT_sb = qk.tile([128, 2, L], _BF16)
            KT_sb = qk.tile([128, 2, L], _BF16)

            def emit_qkt(nt, qk_i):
                x_sb, wbase, dst = ((xq_sb, 0, QT_sb),
                                    (xk_sb, 2, KT_sb))[qk_i]
                lo, hi = nt * 512, (nt + 1) * 512
                ps = psBig.tile([128, 1024], _F32, tag="big")
                for mt in range(2):
                    n = 0
                    for wt in (0, 1):  # W hi, lo; x is hi-only
                        for kp2 in range(2):
                            nc.tensor.matmul(
                                ps[:, mt * 512:(mt + 1) * 512],
                                w8[:, wbase + wt, 2 * kp2:2 * kp2 + 2,
                                   mt * 128:(mt + 1) * 128],
                                x_sb[:, 2 * kp2:2 * kp2 + 2, lo:hi],
                                start=(n == 0), stop=(n == 3), perf_mode=_DR,
                                skip_group_check=True)
                            n += 1
                pv = ps.rearrange("p (mt x) -> p mt x", mt=2)
                if nt == 0:
                    nc.scalar.activation(
                        dst[:, :, lo:hi], pv, ACT.Identity, scale=1.0 / SC_QK)
                else:
                    nc.vector.tensor_scalar(
                        dst[:, :, lo:hi], pv, 1.0 / SC_QK, None, ALU.mult)

            # ---------------- V projection -> Vp [128, NCH, 4*66] (+ones)
            Vp = vpool.tile([128, NCH, 4 * 66], _BF16)
            nc.gpsimd.memset(Vp[:, :, 64::66], 1.0)

            def emit_v2(ltp):
                # lt pair (2*ltp, 2*ltp+1) -> one 2-bank psum, one evac
                ps = psBig.tile([128, 1024], _F32, tag="big")
                for i in range(2):
                    lt = 2 * ltp + i
                    n = 0
                    for xt, wt in ((0, 4), (0, 5), (1, 4)):
                        for kp2 in range(2):
                            nc.tensor.matmul(
                                ps[:, i * 512:i * 512 + 256],
                                xv_sb[:, xt, 2 * kp2:2 * kp2 + 2,
                                      lt * 128:(lt + 1) * 128],
                                w8[:, wt, 2 * kp2:2 * kp2 + 2, :],
                                start=(n == 0), stop=(n == 5), perf_mode=_DR,
                                skip_group_check=True)
                            n += 1
                vdst = Vp[:, 2 * ltp:2 * ltp + 2, :].rearrange(
                    "p t (h x) -> p t h x", h=4)[:, :, :, 0:64]
                vsrc = ps.rearrange("p (t x) -> p t x", t=2)[:, :, 0:256] \
                    .rearrange("p t (h x) -> p t h x", h=4)
                nc.vector.tensor_scalar(vdst, vsrc, 1.0 / SC_QK, None,
                                        ALU.mult)

            # ---------------- state PSUM (persistent, 2 banks) + fp8 shadow
            S_ps = psS.tile([128, 2, 512], _F32, name="S_ps")
            S8_sb = [const.tile([128, 2, 264], _F8, name=f"S8_{i}")
                     for i in range(2)]

            attnT = att.tile([128, 2, L], _BF16)

            _wins = {}

            def alloc_win(w):
                QPw = featq.tile([128, 4, 2, 512], _F8, tag="qw", name=f"qw{w}")
                KPw = featk.tile([128, 4, 2, 512], _F8, tag="kw", name=f"kw{w}")
                _wins[w] = (QPw, KPw)

            def emit_feature_part(w, part):
                """part = (mt, hh) 0..3: one head's q+k features, merged
                2-bank psums, one evac each (prj pre-scaled by SC_F)."""
                QPw, KPw = _wins[w]
                lo, hi = w * 512, (w + 1) * 512
                mt, hh = divmod(part, 2)
                h = 2 * mt + hh
                prj = prjE_sb if hh == 0 else prjO_sb
                psq = psBig.tile([128, 1024], _F32, tag="big")
                psk = psBig.tile([128, 1024], _F32, tag="big")
                for mh in range(2):
                    nc.tensor.matmul(
                        psq[:, mh * 512:(mh + 1) * 512],
                        prj[:, mh * 128:(mh + 1) * 128],
                        QT_sb[:, mt, lo:hi], start=True, stop=True,
                        skip_group_check=True)
                    nc.tensor.matmul(
                        psk[:, mh * 512:(mh + 1) * 512],
                        prj[:, mh * 128:(mh + 1) * 128],
                        KT_sb[:, mt, lo:hi], start=True, stop=True,
                        skip_group_check=True)
                nc.scalar.activation(
                    QPw[:, h, :, :],
                    psq.rearrange("p (mh x) -> p mh x", mh=2), ACT.Relu,
                    bias=fbias(0, h, 0))
                if part % 2 == 0:
                    nc.vector.tensor_scalar(
                        KPw[:, h, :, :],
                        psk.rearrange("p (mh x) -> p mh x", mh=2),
                        fbias(1, h, 0), SC_F * STAB, ALU.add, ALU.max)
                else:
                    nc.scalar.activation(
                        KPw[:, h, :, :],
                        psk.rearrange("p (mh x) -> p mh x", mh=2), ACT.Relu,
                        bias=fbias(1, h, 0))

            kp_store = {}

            def emit_kp(kc):
                # KP natural chunk in [l, feat] layout for dS (bf16, true
                # units; psum is SC_F-scaled, evac divides by SC_F).
                ps = psBig.tile([128, 1024], _F32, tag="big")
                for mt in range(2):
                    nc.tensor.matmul(
                        ps[:, mt * 512:(mt + 1) * 512],
                        KT_sb[:, mt, kc * 128:(kc + 1) * 128],
                        prj_sb, start=True, stop=True, skip_group_check=True)
                kp2 = kpp.tile([128, 1024], _BF16, tag="kp")
                if kc % 2 == 0:
                    nc.vector.tensor_scalar(
                        kp2[:], ps[:], 1.0 / SC_F, STAB, ALU.mult, ALU.max)
                else:
                    nc.scalar.activation(
                        kp2[:], ps[:], ACT.Relu, bias=stab_sb[:],
                        scale=1.0 / SC_F)
                kp_store[kc] = kp2

            # ---------------- prologue: EVERYTHING precomputed (QKT, V,
            # features, kp), interleaved per window so each matmul's DMA
            # dependency just landed; the scan then runs with only its own
            # per-chunk evacs on the vector engines.
            for nt in range(4):
                emit_qkt(nt, 0)
                emit_qkt(nt, 1)
                emit_v2(2 * nt)
                emit_v2(2 * nt + 1)
                alloc_win(nt)
                for part in range(4):
                    emit_feature_part(nt, part)
                for kc in range(4 * nt, 4 * nt + 4):
                    emit_kp(kc)

            # ---------------- scan
            pending = None  # (c_prev, attn4_prev)

            def emit_tail(prev_c, prev_attn4, tail):
                # transposes + attnT evacuation for a finished chunk
                pst = psA.tile([128, 256], _BF16, tag="A", name="pst")
                for mt in range(2):
                    nc.tensor.transpose(
                        pst[:, mt * 128:(mt + 1) * 128],
                        prev_attn4[:, mt * 128:(mt + 1) * 128], id_sb)
                nc.vector.tensor_copy(
                    attnT[:, :, prev_c * 128:(prev_c + 1) * 128],
                    pst[:].rearrange("p (mh x) -> p mh x", mh=2))
                if tail:
                    emit_outproj(prev_c)

            def emit_outproj(prev_c):
                pso = psA.tile([128, 512], _F32, tag="A", name="pso")
                for mh in range(2):
                    nc.tensor.matmul(
                        pso[:],
                        attnT[:, mh, prev_c * 128:(prev_c + 1) * 128],
                        wo_sb[:, mh, :],
                        start=(mh == 0), stop=(mh == 1))
                o_sb = outs.tile([128, 512], _BF16, tag="o")
                nc.scalar.activation(o_sb[:], pso[:], ACT.Copy)
                nc.sync.dma_start(outp[prev_c * 128:(prev_c + 1) * 128, :],
                                  o_sb[:])

            for w in range(NW):
                QPw, KPw = _wins[w]
                for cc in range(NW):
                    c = w * NW + cc
                    cl, ch = cc * 128, (cc + 1) * 128

                    kp2 = kp_store.pop(c)

                    # A~ for ALL 4 heads in one bank [128, 512] (fp8 DR)
                    psa = psA.tile([128, 512], _F32, tag="A", name="psa")
                    for h in range(4):
                        nc.tensor.matmul(
                            psa[:, h * 128:(h + 1) * 128],
                            KPw[:, h, :, cl:ch],
                            QPw[:, h, :, cl:ch],
                            start=(h == 0), stop=(h == 3),
                            perf_mode=_DR, skip_group_check=True)
                    mA4 = small.tile([128, 512], _BF16, tag="mA")
                    nc.vector.tensor_tensor(
                        mA4.rearrange("p (h x) -> p h x", h=4),
                        psa[:].rearrange("p (h x) -> p h x", h=4),
                        mask_sb[:, None, :].to_broadcast([128, 4, 128]),
                        ALU.mult)

                    # dS + its fp8 evac FIRST: the S-state recurrence
                    # (S8(c-1) -> dS(c) -> S8(c)) is the scan's critical
                    # cycle; S8 is double-buffered by chunk parity so the
                    # evac does not wait for this chunk's inter reads.
                    for h in range(4):
                        mt, hh = divmod(h, 2)
                        for mh in range(2):
                            nc.tensor.matmul(
                                S_ps[:, mh, h * 66:h * 66 + 65],
                                kp2[:, mt * 512 + hh * 256 + mh * 128:
                                    mt * 512 + hh * 256 + (mh + 1) * 128],
                                Vp[:, c, h * 66:h * 66 + 65],
                                start=(c == 0 and h == 0),
                                stop=(c == NCH - 1 and h == 3),
                                skip_group_check=True)
                    if c < NCH - 1:
                        nc.scalar.activation(
                            S8_sb[c % 2][:], S_ps[:, :, 0:264], ACT.Copy,
                            scale=SC_S)

                    # previous chunk's transposes fill the mask-wait bubble
                    if pending is not None:
                        emit_tail(*pending, tail=False)

                    # num4 [128, 264]: inter first (fp8 DR), then intra
                    num4 = psA.tile([128, 264], _F32, tag="A", name="num4")
                    if c > 0:
                        for h in range(4):
                            nc.tensor.matmul(
                                num4[:, h * 66:h * 66 + 65],
                                QPw[:, h, :, cl:ch],
                                S8_sb[(c - 1) % 2][:, :, h * 66:h * 66 + 65],
                                start=(h == 0), stop=False,
                                perf_mode=_DR, skip_group_check=True)
                    for h in range(4):
                        nc.tensor.matmul(
                            num4[:, h * 66:h * 66 + 65],
                            mA4[:, h * 128:(h + 1) * 128],
                            Vp[:, c, h * 66:h * 66 + 65],
                            start=(c == 0 and h == 0), stop=(h == 3),
                            skip_group_check=True)

                    # divide all heads at once: attn4 = num/den
                    rd4 = small.tile([128, 4], _F32, tag="rd")
                    if c == 0:
                        rdt = small.tile([128, 4], _F32, tag="rdt")
                        nc.vector.tensor_scalar(
                            rdt[:], num4[:, 64::66], SC_NUM * 1e-6, None,
                            ALU.add)
                        nc.vector.reciprocal(rd4[:], rdt[:])
                    else:
                        nc.vector.reciprocal(rd4[:], num4[:, 64::66])
                    attn4 = small.tile([128, 256], _BF16, tag="attn2")
                    nc.vector.tensor_tensor(
                        attn4.rearrange("p (h x) -> p h x", h=4),
                        num4[:].rearrange("p (h x) -> p h x", h=4)[:, :, 0:64],
                        rd4[:, :, None].to_broadcast([128, 4, 64]),
                        ALU.mult)

                    # previous chunk's out-projection
                    if pending is not None:
                        emit_outproj(pending[0])
                    pending = (c, attn4)

            # flush the last chunk
            emit_tail(*pending, tail=True)

    nc.compile()
    return nc


def _host_prep(inputs):
    """Build per-core in_maps from full inputs."""
    query = np.asarray(inputs["query"], np.float32)
    key = np.asarray(inputs["key"], np.float32)
    value = np.asarray(inputs["value"], np.float32)
    proj = np.asarray(inputs["proj"], np.float32)
    w_q_w = np.asarray(inputs["w_q_w"], np.float32)
    w_q_b = np.asarray(inputs["w_q_b"], np.float32)
    w_k_w = np.asarray(inputs["w_k_w"], np.float32)
    w_k_b = np.asarray(inputs["w_k_b"], np.float32)
    w_v_w = np.asarray(inputs["w_v_w"], np.float32)
    w_o_w = np.asarray(inputs["w_o_w"], np.float32)

    def hilo8(a, scale):
        s = a * scale
        hi = s.astype(_NP_F8)
        lo = (s - hi.astype(np.float32)).astype(_NP_F8)
        return hi, lo

    # x tensors are shared across the core pairs: quantize once
    x8 = {}
    for nm, arr in (("q", query), ("k", key)):
        x8[nm] = [np.ascontiguousarray(arr[b].T * SC_X).astype(_NP_F8)
                  for b in range(B)]
    per_b = []
    for b in range(B):
        hi, lo = hilo8(np.ascontiguousarray(value[b].T), SC_X)
        per_b.append(np.stack([hi, lo], axis=0))  # [2, DIN, L]
    x8["v"] = per_b

    in_maps = []
    for core in range(N_CORES):
        b, hg = divmod(core, 2)
        hsl = slice(hg * 256, (hg + 1) * 256)

        c8 = np.zeros((128, 6144), _NP_F8)
        for wi, wmat in ((0, w_q_w), (2, w_k_w), (4, w_v_w)):
            wT = wmat[hsl].T  # [512, 256]
            hi, lo = hilo8(wT, SC_W)
            for ko in range(4):
                base = wi * 1024 + ko * 256
                c8[:, base:base + 256] = hi[ko * 128:(ko + 1) * 128]
                c8[:, base + 1024:base + 1280] = lo[ko * 128:(ko + 1) * 128]

        cdt = np.zeros((128, _W_CDT), np.float32)
        prj_s = proj.T * RATIO * SC_F
        cdt[0:64, _OFF_PRJ:_OFF_PRJ + 256] = prj_s
        cdt[64:128, _OFF_PRJ + 256:_OFF_PRJ + 512] = prj_s
        woT = w_o_w[:, hsl].T  # [256, 512]
        for mh in range(2):
            cdt[:, _OFF_WO + mh * 512:_OFF_WO + (mh + 1) * 512] = \
                woT[mh * 128:(mh + 1) * 128]
        cdt[:, _OFF_ID:_OFF_ID + 128] = np.eye(128, dtype=np.float32)

        cf = np.zeros((128, _W_CF), np.float32)
        cf[:, 0:128] = np.triu(np.ones((128, 128), np.float32)) * MASKC
        # feature bias per (q/k, head, mh): SC_F * (bias_h @ projT + stab)
        pmat = proj.T * RATIO  # [64, 256]
        for qk_i, bias in ((0, w_q_b), (1, w_k_b)):
            for h in range(NH):
                bh = bias[hsl][h * 64:(h + 1) * 64]  # [64]
                bf = (bh @ pmat) * SC_F + SC_F * STAB  # [256]
                for mh in range(2):
                    cf[:, 128 + qk_i * 8 + h * 2 + mh] = \
                        bf[mh * 128:(mh + 1) * 128]

        m = {
            "xq8": x8["q"][b],
            "xk8": x8["k"][b],
            "xv8": x8["v"][b],
            "cfp8": c8,
            "cdt": cdt.astype(_NP_BF16),
            "cf32": cf,
        }
        in_maps.append(m)
    return in_maps


def kernel(**inputs):
    if "nc" not in _CACHED:
        _CACHED["nc"] = _build_nc()
    nc = _CACHED["nc"]

    in_maps = _host_prep(inputs)
    res = bass_utils.run_bass_kernel_spmd(
        nc, in_maps, core_ids=list(range(N_CORES)))

    w_v_b = np.asarray(inputs["w_v_b"], np.float32)
    w_o_w = np.asarray(inputs["w_o_w"], np.float32)
    w_o_b = np.asarray(inputs["w_o_b"], np.float32)

    out = np.zeros((B, L, DIN), np.float32)
    for core in range(N_CORES):
        b, hg = divmod(core, 2)
        out[b] += res.results[core]["outp"].astype(np.float32)
    # v-bias enters attn additively per dh slice: out += vb @ WoT (+ out bias)
    out += (w_v_b[None, :] @ w_o_w.T)[0][None, None, :]
    out += w_o_b[None, None, :]
    return out

